# revision 1
# baseline (speedup 1.0000x reference)
"""AttentionBlock (GroupNorm + single-head spatial attention + proj + residual)
on 8 trn2 NeuronCores, data-parallel over the batch (1 image per core).

Full inputs in, full outputs out. Layouts are prepared host-side so every DMA
is contiguous and the device program needs no transposes:
  - activations live as [128 part, ct, pix]  (channel tiles of 128)
  - weights are passed pre-transposed as [c_in part, ct_in, c_out]
  - v is produced directly transposed (v^T = xn^T @ W_v^T) so the
    attention contraction over pixels has pixels on partitions everywhere.

Matmuls run in float32r (single-pass fp32 PE mode, ~0.28us per
128x128x512 tile at 2.4GHz; measured end-to-end rel err vs fp32 ~2e-5).
A burst of throwaway matmuls at kernel start keeps the PE busy through the
input-DMA window so the HAM clock gate reaches 8/8 before the real GEMMs.
"""

import sys

sys.path.insert(0, "/opt/trn_rl_repo")

import numpy as np

import concourse.bass as bass
import concourse.tile as tile
from concourse import bacc, mybir
from concourse.bass_utils import run_bass_kernel_spmd
from concourse.tile_rust import add_dep_helper

F32 = mybir.dt.float32
F32R = mybir.dt.float32r

C = 512          # channels
NPIX = 1024      # pixels per image (32*32)
CT = 4           # channel tiles of 128
JT = 8           # pixel tiles of 128
NH = 2           # halves of NPIX for the 512-wide moving dim
G = 32           # groups
GS = 16          # channels per group
EPS = 1e-5
SCALE = C ** -0.5
WARM_MMS = 38    # PE warm-up matmuls during the input-DMA window

# matmul dtype for the big GEMMs: float32r streams one 512-wide tile in
# ~280ns warm (vs ~1.1us for 2-pass fp32).  Set to F32 for full precision.
MM_DT = F32R

TRACE = False          # set True (from test.py) to capture an NTFF profile
TRACE_KW = {}          # extra kwargs for run_bass_kernel_spmd
LAST_RESULTS = None    # BassKernelResults of the most recent run

_cache = {}


def _build(fold_qk=True):
    nc = bacc.Bacc("TRN2")

    x_d = nc.dram_tensor("x", [128, CT, NPIX], F32, kind="ExternalInput")
    qwcols = 2 * C if fold_qk else 3 * C
    qw_d = nc.dram_tensor("qw", [128, CT, qwcols], MM_DT, kind="ExternalInput")
    pw_d = nc.dram_tensor("pw", [128, CT, C], MM_DT, kind="ExternalInput")
    gnw_d = nc.dram_tensor("gnw", [128, CT], F32, kind="ExternalInput")
    gnb_d = nc.dram_tensor("gnb", [128, CT], F32, kind="ExternalInput")
    if not fold_qk:
        qb_d = nc.dram_tensor("qb", [128, CT], F32, kind="ExternalInput")
        kb_d = nc.dram_tensor("kb", [128, CT], F32, kind="ExternalInput")
    pb_d = nc.dram_tensor("pb", [128, CT], F32, kind="ExternalInput")
    y_d = nc.dram_tensor("y", [128, CT, NPIX], F32, kind="ExternalOutput")

    # Indicator constants for the cross-partition group reductions.
    # ind1[p, ct*G + g] = 1 iff channel (ct*128+p) belongs to group g.
    ind1 = np.zeros((128, CT * G), np.float32)
    for ct in range(CT):
        for p in range(128):
            ind1[p, ct * G + ct * 8 + p // GS] = 1.0
    # ind2[g, c] = 1 iff channel c belongs to group g.
    ind2 = np.zeros((G, C), np.float32)
    for c in range(C):
        ind2[c // GS, c] = 1.0
    ind1_d = nc.inline_tensor(ind1, name="ind1")
    ind2_d = nc.inline_tensor(ind2, name="ind2")
    onesc_d = nc.dram_tensor("onesc", [128, 512], MM_DT, kind="ExternalInput")
    onesr_d = nc.dram_tensor("onesr", [1, 128], MM_DT, kind="ExternalInput")

    with tile.TileContext(nc) as tc:
        with (
            nc.allow_low_precision(reason="float32r matmul operands (4B fp32 bits)"),
            tc.tile_pool(name="persist", bufs=1) as pers,
            tc.tile_pool(name="small", bufs=4) as spool,
            tc.tile_pool(name="ps", bufs=8, space="PSUM") as psp,
        ):
            # ---- warm-up source first, then x gets the DMA bandwidth -------
            onesc_sb = pers.tile([128, 512], MM_DT)
            nc.sync.dma_start(onesc_sb[:], onesc_d[:])
            ones_col = onesc_sb[:, 0:1]

            # ---- x, one DMA per (ct, half) so bn_stats overlaps ------------
            x_sb = pers.tile([128, CT, NPIX], F32)
            x_dmas = []
            for ct in range(CT):
                for nh in range(NH):
                    x_dmas.append(
                        nc.sync.dma_start(
                            x_sb[:, ct, nh * 512 : (nh + 1) * 512],
                            x_d[:, ct, nh * 512 : (nh + 1) * 512],
                        )
                    )

            # ---- tiny loads (after x in the issue queue) -------------------
            gnw_sb = pers.tile([128, CT], F32)
            nc.sync.dma_start(gnw_sb[:], gnw_d[:])
            gnb_sb = pers.tile([128, CT], F32)
            nc.sync.dma_start(gnb_sb[:], gnb_d[:])
            ind1_sb = pers.tile([128, CT * G], F32)
            nc.sync.dma_start(ind1_sb[:], ind1_d[:])
            ind2_sb = pers.tile([G, C], F32)
            nc.sync.dma_start(ind2_sb[:], ind2_d[:])
            if not fold_qk:
                qb_sb = pers.tile([128, CT], F32)
                nc.sync.dma_start(qb_sb[:], qb_d[:])
                kb_sb = pers.tile([128, CT], F32)
                nc.sync.dma_start(kb_sb[:], kb_d[:])
            pb_sb = pers.tile([128, CT], F32)
            nc.sync.dma_start(pb_sb[:], pb_d[:])
            ones_row = pers.tile([1, 128], MM_DT)
            nc.sync.dma_start(ones_row[:], onesr_d[:])

            # ---- weights, serialized behind x so x gets the DMA bandwidth --
            qw_sb = pers.tile([128, CT, qwcols], MM_DT)
            for ci in range(CT):
                d = nc.sync.dma_start(qw_sb[:, ci, :], qw_d[:, ci, :])
                add_dep_helper(d.ins, x_dmas[-1].ins, sync=True,
                               reason="let x DMA finish first")
            pw_sb = pers.tile([128, CT, C], MM_DT)
            d = nc.sync.dma_start(pw_sb[:], pw_d[:])
            add_dep_helper(d.ins, x_dmas[-1].ins, sync=True,
                           reason="let x DMA finish first")

            eps_sb = pers.tile([G, 1], F32)
            nc.vector.memset(eps_sb[:], EPS)
            ones_row32 = pers.tile([1, 128], F32)
            nc.vector.memset(ones_row32[:], 1.0)
            ones_col32 = pers.tile([128, 1], F32)
            nc.vector.memset(ones_col32[:], 1.0)

            # ---- PE warm-up: keep HAM busy while the input DMAs stream -----
            warm_ps = psp.tile([128, 512], F32, tag="ps")
            for _ in range(WARM_MMS):
                nc.tensor.matmul(
                    warm_ps[:], onesc_sb[:, 0:128], onesc_sb[:], start=True, stop=True
                )

            # ---- group norm ------------------------------------------------
            # per-channel mean / E[x^2] along pixels, then group-combine via
            # indicator matmuls (contraction over the partition dim).
            statcols = pers.tile([128, CT, 2], F32)
            for ct in range(CT):
                st6 = spool.tile([128, 2, 6], F32, tag="st6")
                nc.vector.bn_stats(st6[:, 0, :], x_sb[:, ct, 0:512])
                nc.vector.bn_stats(st6[:, 1, :], x_sb[:, ct, 512:1024])
                mv = spool.tile([128, 2], F32, tag="mv")
                nc.vector.bn_aggr(mv[:], st6[:])
                nc.vector.tensor_copy(statcols[:, ct, 0:1], mv[:, 0:1])
                # E[x^2] = var + mean^2
                nc.vector.tensor_mul(statcols[:, ct, 1:2], mv[:, 0:1], mv[:, 0:1])
                nc.vector.tensor_add(
                    statcols[:, ct, 1:2], statcols[:, ct, 1:2], mv[:, 1:2]
                )

            gsum_ps = psp.tile([G, 2], F32, tag="ps")
            for ct in range(CT):
                nc.tensor.matmul(
                    gsum_ps[:],
                    ind1_sb[:, ct * G : (ct + 1) * G],
                    statcols[:, ct, :],
                    start=(ct == 0),
                    stop=(ct == CT - 1),
                )
            gs_sb = spool.tile([G, 2], F32, tag="gs")
            nc.vector.tensor_scalar_mul(gs_sb[:], gsum_ps[:], 1.0 / GS)
            var32 = spool.tile([G, 1], F32, tag="var32")
            nc.vector.tensor_mul(var32[:], gs_sb[:, 0:1], gs_sb[:, 0:1])
            nc.vector.tensor_sub(var32[:], gs_sb[:, 1:2], var32[:])
            # grow = [rstd, mean * rstd] per group
            grow = pers.tile([G, 2], F32)
            nc.scalar.activation(
                grow[:, 0:1],
                var32[:],
                mybir.ActivationFunctionType.Sqrt,
                bias=eps_sb[:],
            )
            nc.vector.reciprocal(grow[:, 0:1], grow[:, 0:1])
            nc.vector.tensor_mul(grow[:, 1:2], gs_sb[:, 0:1], grow[:, 0:1])

            # broadcast group stats back to channels; fold gn weight/bias into
            # per-channel scale A and bias B:  xn = x*A + B
            xn_sb = pers.tile([128, CT, NPIX], MM_DT)
            chsb = pers.tile([128, CT, 2], F32)
            for ct in range(CT):
                bc_ps = psp.tile([128, 2], F32, tag="ps")
                nc.tensor.matmul(
                    bc_ps[:],
                    ind2_sb[:, ct * 128 : (ct + 1) * 128],
                    grow[:],
                    start=True,
                    stop=True,
                )
                nc.vector.tensor_mul(
                    chsb[:, ct, 0:1], gnw_sb[:, ct : ct + 1], bc_ps[:, 0:1]
                )
                nc.vector.tensor_mul(
                    chsb[:, ct, 1:2], gnw_sb[:, ct : ct + 1], bc_ps[:, 1:2]
                )
                nc.vector.tensor_sub(
                    chsb[:, ct, 1:2], gnb_sb[:, ct : ct + 1], chsb[:, ct, 1:2]
                )
                nc.vector.tensor_scalar(
                    out=xn_sb[:, ct, :],
                    in0=x_sb[:, ct, :],
                    scalar1=chsb[:, ct, 0:1],
                    scalar2=chsb[:, ct, 1:2],
                    op0=mybir.AluOpType.mult,
                    op1=mybir.AluOpType.add,
                )

            # ---- queries/keys --------------------------------------------
            # fold path: t = A @ xn with A = W_q^T W_k (host-precomputed);
            # S^T = xn^T t then equals k^T q up to softmax-invariant terms.
            # legacy path: explicit q, k with their biases.
            if fold_qk:
                t_sb = pers.tile([128, CT, NPIX], MM_DT)
                for co in range(CT):
                    for nh in range(NH):
                        ps = psp.tile([128, 512], F32, tag="ps")
                        for ci in range(CT):
                            nc.tensor.matmul(
                                ps[:],
                                qw_sb[:, ci, co * 128 : (co + 1) * 128],
                                xn_sb[:, ci, nh * 512 : (nh + 1) * 512],
                                start=(ci == 0),
                                stop=(ci == CT - 1),
                            )
                        nc.scalar.activation(
                            t_sb[:, co, nh * 512 : (nh + 1) * 512],
                            ps[:],
                            mybir.ActivationFunctionType.Identity,
                        )
                q_sb = t_sb
                k_sb = xn_sb
            else:
                q_sb = pers.tile([128, CT, NPIX], MM_DT)
                k_sb = pers.tile([128, CT, NPIX], MM_DT)
                for dst, wofs, b_sb in ((q_sb, 0, qb_sb), (k_sb, C, kb_sb)):
                    for co in range(CT):
                        for nh in range(NH):
                            ps = psp.tile([128, 512], F32, tag="ps")
                            for ci in range(CT):
                                nc.tensor.matmul(
                                    ps[:],
                                    qw_sb[:, ci, wofs + co * 128 : wofs + (co + 1) * 128],
                                    xn_sb[:, ci, nh * 512 : (nh + 1) * 512],
                                    start=(ci == 0),
                                    stop=(ci == CT - 1),
                                )
                            nc.scalar.activation(
                                dst[:, co, nh * 512 : (nh + 1) * 512],
                                ps[:],
                                mybir.ActivationFunctionType.Identity,
                                bias=b_sb[:, co : co + 1],
                            )

            # ---- v^T = xn^T @ W_v^T + 1 x b_v  (out: [pix part, c_out]) ----
            vt_sb = pers.tile([128, JT, C], MM_DT)
            for jt in range(JT):
                ps = psp.tile([128, 512], F32, tag="ps")
                for ci in range(CT):
                    nc.tensor.matmul(
                        ps[:],
                        xn_sb[:, ci, jt * 128 : (jt + 1) * 128],
                        qw_sb[:, ci, qwcols - C : qwcols],
                        start=(ci == 0),
                        stop=(ci == CT - 1),
                    )
                nc.vector.tensor_copy(vt_sb[:, jt, :], ps[:])

            # ---- S^T = k^T q (pix_j on partitions), E = exp(scale * S^T) ---
            # ih-major: each half's denominator + reciprocal hides under the
            # other half's matmuls.
            e_sb = pers.tile([128, JT, NPIX], MM_DT)
            recip_sb = pers.tile([1, NPIX], F32)
            for nh in range(NH):
                dps = psp.tile([1, 512], F32, name=f"den{nh}", tag="ps")
                for jt in range(JT):
                    ps = psp.tile([128, 512], F32, tag="ps")
                    for ci in range(CT):
                        nc.tensor.matmul(
                            ps[:],
                            k_sb[:, ci, jt * 128 : (jt + 1) * 128],
                            q_sb[:, ci, nh * 512 : (nh + 1) * 512],
                            start=(ci == 0),
                            stop=(ci == CT - 1),
                        )
                    esl = e_sb[:, jt, nh * 512 : (nh + 1) * 512]
                    nc.scalar.activation(
                        esl, ps[:], mybir.ActivationFunctionType.Exp, scale=SCALE
                    )
                    nc.tensor.matmul(
                        dps[:],
                        ones_col,
                        esl,
                        start=(jt == 0),
                        stop=(jt == JT - 1),
                    )
                rsl = recip_sb[0:1, nh * 512 : (nh + 1) * 512]
                rscr = spool.tile([1, 512], F32, tag="rscr")
                nc.vector.reciprocal_approx_accurate(rsl, dps[:], rscr[:])

            # ---- broadcast 1/denom across partitions (fp32 matmul) ---------
            rb_sb = pers.tile([128, NPIX], F32)

            def bcast_recip(nh):
                bp = psp.tile([128, 512], F32, name=f"bp{nh}", tag="ps")
                nc.tensor.matmul(
                    bp[:],
                    ones_row32[0:1, :],
                    recip_sb[0:1, nh * 512 : (nh + 1) * 512],
                    start=True,
                    stop=True,
                )
                nc.scalar.activation(
                    rb_sb[:, nh * 512 : (nh + 1) * 512],
                    bp[:],
                    mybir.ActivationFunctionType.Identity,
                )

            # ---- att = v^T^T @ E, normalized on evacuation -----------------
            att_sb = pers.tile([128, CT, NPIX], MM_DT)
            for nh in range(NH):
                bcast_recip(nh)
                for ct in range(CT):
                    ps = psp.tile([128, 512], F32, tag="ps")
                    for jt in range(JT):
                        nc.tensor.matmul(
                            ps[:],
                            vt_sb[:, jt, ct * 128 : (ct + 1) * 128],
                            e_sb[:, jt, nh * 512 : (nh + 1) * 512],
                            start=(jt == 0),
                            stop=(jt == JT - 1),
                        )
                    nc.vector.tensor_mul(
                        att_sb[:, ct, nh * 512 : (nh + 1) * 512],
                        ps[:],
                        rb_sb[:, nh * 512 : (nh + 1) * 512],
                    )

            # ---- out = proj_w @ att + proj_b + x, streamed to DRAM ---------
            for co in range(CT):
                for nh in range(NH):
                    ps = psp.tile([128, 512], F32, tag="ps")
                    for ci in range(CT):
                        nc.tensor.matmul(
                            ps[:],
                            pw_sb[:, ci, co * 128 : (co + 1) * 128],
                            att_sb[:, ci, nh * 512 : (nh + 1) * 512],
                            start=(ci == 0),
                            stop=(ci == CT - 1),
                        )
                    sl = (slice(None), co, slice(nh * 512, (nh + 1) * 512))
                    nc.vector.scalar_tensor_tensor(
                        out=x_sb[sl],
                        in0=ps[:],
                        scalar=pb_sb[:, co : co + 1],
                        in1=x_sb[sl],
                        op0=mybir.AluOpType.add,
                        op1=mybir.AluOpType.add,
                    )
                    nc.sync.dma_start(y_d[sl], x_sb[sl])

    nc.compile()
    return nc


def kernel(x, gn_weight, gn_bias, qkv_w, qkv_b, proj_w, proj_b):
    global LAST_RESULTS
    b, c, h, w = x.shape
    assert (b, c, h * w) == (8, C, NPIX)

    qkv_b = np.asarray(qkv_b, np.float32)
    qkv_w = np.asarray(qkv_w, np.float32)
    proj_w = np.asarray(proj_w, np.float32)
    # The per-query bias term cancels in softmax; a nonzero q-bias would
    # contribute a per-key term, so only then fall back to explicit q/k.
    fold_qk = not np.any(qkv_b[0:C])

    if ("nc", fold_qk) not in _cache:
        _cache[("nc", fold_qk)] = _build(fold_qk)
    nc = _cache[("nc", fold_qk)]

    def col(v):  # [512] vector -> [128, CT] per-partition columns
        return np.ascontiguousarray(np.asarray(v, np.float32).reshape(CT, 128).T)

    def wtile(wT, cols):  # [c_in, cols] -> [128, CT, cols]
        return np.ascontiguousarray(
            np.asarray(wT, np.float32).reshape(CT, 128, cols).transpose(1, 0, 2)
        )

    if fold_qk:
        # A^T = W_q^T W_k in fp64, laid out like a weight: lhsT[b, a]
        At = (qkv_w[0:C].astype(np.float64).T @ qkv_w[C : 2 * C].astype(np.float64))
        qw_host = np.concatenate(
            [At.astype(np.float32), qkv_w[2 * C :].T], axis=1
        )  # [c_in, 2C]
    else:
        qw_host = qkv_w.T  # [c_in, 3C]

    shared = {
        "qw": wtile(qw_host, qw_host.shape[1]),
        "pw": wtile(proj_w.T, C),
        "gnw": col(gn_weight),
        "gnb": col(gn_bias),
        # attention rows sum to 1, so att(v + b_v) = att(v) + b_v; fold the
        # v bias through proj into the proj bias on the host.
        "pb": col(proj_b + proj_w @ qkv_b[2 * C :]),
        "onesc": np.ones((128, 512), np.float32),
        "onesr": np.ones((1, 128), np.float32),
    }
    if not fold_qk:
        shared["qb"] = col(qkv_b[0:C])
        shared["kb"] = col(qkv_b[C : 2 * C])
    xs = np.asarray(x, np.float32).reshape(b, CT, 128, NPIX)
    in_maps = [
        {"x": np.ascontiguousarray(xs[i].transpose(1, 0, 2)), **shared}
        for i in range(b)
    ]

    res = run_bass_kernel_spmd(
        nc, in_maps, core_ids=list(range(8)), trace=TRACE, **TRACE_KW
    )
    LAST_RESULTS = res
    out = np.stack(
        [r["y"].transpose(1, 0, 2).reshape(c, h, w) for r in res.results]
    )
    return out.astype(np.float32)



# revision 4
# speedup vs baseline: 1.3741x; 1.3741x over previous
"""AttentionBlock (GroupNorm + single-head spatial attention + proj + residual)
on 8 trn2 NeuronCores, data-parallel over the batch (1 image per core).

fp8 build: all five GEMM groups + the softmax-denominator reduction run as
fp8e4m3 DoubleRow matmuls (K=256 per instruction), halving PE instruction
count vs float32r. Host-side scaling keeps operands in e4m3's normal range:
  - A = W_k^T W_q and W_v are scaled x16 (entries ~N(0,1/512) otherwise sit
    in the subnormal band); the x16 on q cancels via exp scale /16, the x16
    on v cancels against the denominator's x16 ones-vector.
  - E = exp(logits - 1.5): the -1.5 shift is softmax-invariant and keeps
    exp() comfortably under e4m3's 448 max.
GroupNorm rstd uses exp(-0.5*ln(var+eps)) so the whole kernel needs a single
activation table (natural_log_exp_and_others: ln, exp, identity).
Measured end-to-end rel err vs fp32 reference ~5e-3 (budget 2e-2).
"""

import sys

sys.path.insert(0, "/opt/trn_rl_repo")

import numpy as np
import ml_dtypes

import concourse.bass as bass
import concourse.tile as tile
from concourse import bacc, mybir
from concourse.bass_utils import run_bass_kernel_spmd
from concourse.tile_rust import add_dep_helper

F32 = mybir.dt.float32
F32R = mybir.dt.float32r
BF16 = mybir.dt.bfloat16
FP8 = mybir.dt.float8e4
DR = mybir.MatmulPerfMode.DoubleRow

C = 512          # channels
NPIX = 1024      # pixels per image (32*32)
CT = 4           # channel tiles of 128
JT = 8           # pixel tiles of 128
NH = 2           # halves of NPIX for the 512-wide moving dim
G = 32           # groups
GS = 16          # channels per group
EPS = 1e-5
SCALE = C ** -0.5
WSCALE = 16.0    # host-side scale on A and W_v (and the den ones-vector)
ESHIFT = -1.5    # softmax-invariant logit shift keeping exp() in fp8 range
WARM_MMS = 40    # PE warm-up matmuls during the input-DMA window

TRACE = False          # set True (from test.py) to capture an NTFF profile
TRACE_KW = {}          # extra kwargs for run_bass_kernel_spmd
LAST_RESULTS = None    # BassKernelResults of the most recent run

_cache = {}


def _build_fp8():
    nc = bacc.Bacc("TRN2")

    x_d = nc.dram_tensor("x", [128, CT, NPIX], F32, kind="ExternalInput")
    qw_d = nc.dram_tensor("qw", [128, CT, 2 * C], FP8, kind="ExternalInput")
    pw_d = nc.dram_tensor("pw", [128, CT, C], FP8, kind="ExternalInput")
    gnw_d = nc.dram_tensor("gnw", [128, CT], F32, kind="ExternalInput")
    gnb_d = nc.dram_tensor("gnb", [128, CT], F32, kind="ExternalInput")
    pb_d = nc.dram_tensor("pb", [128, CT], F32, kind="ExternalInput")
    y_d = nc.dram_tensor("y", [128, CT, NPIX], F32, kind="ExternalOutput")

    # Indicator constants for the cross-partition group reductions.
    ind1 = np.zeros((128, CT * G), np.float32)
    for ct in range(CT):
        for p in range(128):
            ind1[p, ct * G + ct * 8 + p // GS] = 1.0
    ind2 = np.zeros((G, C), np.float32)
    for c in range(C):
        ind2[c // GS, c] = 1.0
    ind1_d = nc.inline_tensor(ind1, name="ind1")
    ind2_d = nc.inline_tensor(ind2, name="ind2")
    ones8_d = nc.dram_tensor("ones8", [128, 2, 512], FP8, kind="ExternalInput")
    onesr_d = nc.dram_tensor("onesr", [1, 128], BF16, kind="ExternalInput")

    with tile.TileContext(nc) as tc:
        with (
            nc.allow_low_precision(reason="fp8 matmul pipeline, validated 5e-3"),
            tc.tile_pool(name="persist", bufs=1) as pers,
            tc.tile_pool(name="small", bufs=4) as spool,
            tc.tile_pool(name="ps", bufs=6, space="PSUM") as psp,
            tc.tile_pool(name="psden", bufs=2, space="PSUM") as psd,
        ):
            # ---- warm-up source first, then x gets the DMA bandwidth -------
            ones8_sb = pers.tile([128, 2, 512], FP8)
            nc.sync.dma_start(ones8_sb[:], ones8_d[:])

            # ---- x, one DMA per (ct, half) so bn_stats overlaps ------------
            x_sb = pers.tile([128, CT, NPIX], F32)
            x_dmas = []
            for ct in range(CT):
                for nh in range(NH):
                    x_dmas.append(
                        nc.sync.dma_start(
                            x_sb[:, ct, nh * 512 : (nh + 1) * 512],
                            x_d[:, ct, nh * 512 : (nh + 1) * 512],
                        )
                    )

            # ---- tiny loads (after x in the issue queue) -------------------
            gnw_sb = pers.tile([128, CT], F32)
            nc.sync.dma_start(gnw_sb[:], gnw_d[:])
            gnb_sb = pers.tile([128, CT], F32)
            nc.sync.dma_start(gnb_sb[:], gnb_d[:])
            ind1_sb = pers.tile([128, CT * G], F32)
            nc.sync.dma_start(ind1_sb[:], ind1_d[:])
            ind2_sb = pers.tile([G, C], F32)
            nc.sync.dma_start(ind2_sb[:], ind2_d[:])
            pb_sb = pers.tile([128, CT], F32)
            nc.sync.dma_start(pb_sb[:], pb_d[:])
            ones_row = pers.tile([1, 128], BF16)
            nc.sync.dma_start(ones_row[:], onesr_d[:])

            # ---- weights, serialized behind x so x gets the DMA bandwidth --
            qw_sb = pers.tile([128, CT, 2 * C], FP8)
            d = nc.sync.dma_start(qw_sb[:, :, 0:C], qw_d[:, :, 0:C])
            add_dep_helper(d.ins, x_dmas[-1].ins, sync=True,
                           reason="let x DMA finish first")
            d = nc.sync.dma_start(qw_sb[:, :, C : 2 * C], qw_d[:, :, C : 2 * C])
            add_dep_helper(d.ins, x_dmas[-1].ins, sync=True,
                           reason="let x DMA finish first")
            pw_sb = pers.tile([128, CT, C], FP8)
            d = nc.sync.dma_start(pw_sb[:], pw_d[:])
            add_dep_helper(d.ins, x_dmas[-1].ins, sync=True,
                           reason="let x DMA finish first")

            eps_sb = pers.tile([G, 1], F32)
            nc.vector.memset(eps_sb[:], EPS)
            eshift_sb = pers.tile([128, 1], F32)
            nc.vector.memset(eshift_sb[:], ESHIFT)

            # ---- PE warm-up: keep HAM busy while the input DMAs stream -----
            warm_ps = psp.tile([128, 512], F32, tag="ps")
            for _ in range(WARM_MMS):
                nc.tensor.matmul(
                    warm_ps[:],
                    ones8_sb[:, 0:2, 0:128],
                    ones8_sb[:, 0:2, :],
                    start=True,
                    stop=True,
                    perf_mode=DR,
                )

            # ---- group norm ------------------------------------------------
            statcols = pers.tile([128, CT, 2], F32)
            for ct in range(CT):
                st6 = spool.tile([128, 2, 6], F32, tag="st6")
                nc.vector.bn_stats(st6[:, 0, :], x_sb[:, ct, 0:512])
                nc.vector.bn_stats(st6[:, 1, :], x_sb[:, ct, 512:1024])
                mv = spool.tile([128, 2], F32, tag="mv")
                nc.vector.bn_aggr(mv[:], st6[:])
                nc.vector.tensor_copy(statcols[:, ct, 0:1], mv[:, 0:1])
                # E[x^2] = var + mean^2
                nc.vector.tensor_mul(statcols[:, ct, 1:2], mv[:, 0:1], mv[:, 0:1])
                nc.vector.tensor_add(
                    statcols[:, ct, 1:2], statcols[:, ct, 1:2], mv[:, 1:2]
                )

            gsum_ps = psp.tile([G, 2], F32, tag="ps")
            for ct in range(CT):
                nc.tensor.matmul(
                    gsum_ps[:],
                    ind1_sb[:, ct * G : (ct + 1) * G],
                    statcols[:, ct, :],
                    start=(ct == 0),
                    stop=(ct == CT - 1),
                )
            gs_sb = spool.tile([G, 2], F32, tag="gs")
            nc.vector.tensor_scalar_mul(gs_sb[:], gsum_ps[:], 1.0 / GS)
            var32 = spool.tile([G, 1], F32, tag="var32")
            nc.vector.tensor_mul(var32[:], gs_sb[:, 0:1], gs_sb[:, 0:1])
            nc.vector.tensor_sub(var32[:], gs_sb[:, 1:2], var32[:])
            # rstd = exp(-0.5 * ln(var+eps)): ln+exp live in the same act
            # table as the softmax Exp, so the kernel loads one table total.
            grow = pers.tile([G, 2], F32)
            lnv = spool.tile([G, 1], F32, tag="lnv")
            nc.scalar.activation(
                lnv[:], var32[:], mybir.ActivationFunctionType.Ln, bias=eps_sb[:]
            )
            nc.scalar.activation(
                grow[:, 0:1], lnv[:], mybir.ActivationFunctionType.Exp, scale=-0.5
            )
            nc.vector.tensor_mul(grow[:, 1:2], gs_sb[:, 0:1], grow[:, 0:1])

            # broadcast group stats back to channels; fold gn weight/bias into
            # per-channel scale A and bias B:  xn = x*A + B  (cast to fp8)
            xn_sb = pers.tile([128, CT, NPIX], FP8)
            chsb = pers.tile([128, CT, 2], F32)
            for ct in range(CT):
                bc_ps = psp.tile([128, 2], F32, tag="ps")
                nc.tensor.matmul(
                    bc_ps[:],
                    ind2_sb[:, ct * 128 : (ct + 1) * 128],
                    grow[:],
                    start=True,
                    stop=True,
                )
                nc.vector.tensor_mul(
                    chsb[:, ct, 0:1], gnw_sb[:, ct : ct + 1], bc_ps[:, 0:1]
                )
                nc.vector.tensor_mul(
                    chsb[:, ct, 1:2], gnw_sb[:, ct : ct + 1], bc_ps[:, 1:2]
                )
                nc.vector.tensor_sub(
                    chsb[:, ct, 1:2], gnb_sb[:, ct : ct + 1], chsb[:, ct, 1:2]
                )
                nc.vector.tensor_scalar(
                    out=xn_sb[:, ct, :],
                    in0=x_sb[:, ct, :],
                    scalar1=chsb[:, ct, 0:1],
                    scalar2=chsb[:, ct, 1:2],
                    op0=mybir.AluOpType.mult,
                    op1=mybir.AluOpType.add,
                )

            # ---- t = (16A) @ xn  (q-path, fp8 DoubleRow) -------------------
            t_sb = pers.tile([128, CT, NPIX], FP8)
            for co in range(CT):
                for nh in range(NH):
                    ps = psp.tile([128, 512], F32, tag="ps")
                    for i in range(2):
                        nc.tensor.matmul(
                            ps[:],
                            qw_sb[:, 2 * i : 2 * i + 2, co * 128 : (co + 1) * 128],
                            xn_sb[:, 2 * i : 2 * i + 2, nh * 512 : (nh + 1) * 512],
                            start=(i == 0),
                            stop=(i == 1),
                            perf_mode=DR,
                        )
                    nc.scalar.activation(
                        t_sb[:, co, nh * 512 : (nh + 1) * 512],
                        ps[:],
                        mybir.ActivationFunctionType.Identity,
                    )

            # ---- v^T = xn^T @ (16 W_v^T)  (out: [pix part, c_out]) ---------
            vt_sb = pers.tile([128, JT, C], FP8)
            for jt in range(JT):
                ps = psp.tile([128, 512], F32, tag="ps")
                for i in range(2):
                    nc.tensor.matmul(
                        ps[:],
                        xn_sb[:, 2 * i : 2 * i + 2, jt * 128 : (jt + 1) * 128],
                        qw_sb[:, 2 * i : 2 * i + 2, C : 2 * C],
                        start=(i == 0),
                        stop=(i == 1),
                        perf_mode=DR,
                    )
                nc.vector.tensor_copy(vt_sb[:, jt, :], ps[:])

            # ---- S^T = k^T q, E = exp(scale/16 * S^T - 1.5), den ----------
            e_sb = pers.tile([128, JT, NPIX], FP8)
            recip_sb = pers.tile([1, NPIX], F32)
            recip_bf = pers.tile([1, NPIX], BF16)
            dps_tiles = []
            for nh in range(NH):
                for jt in range(JT):
                    ps = psp.tile([128, 512], F32, tag="ps")
                    for i in range(2):
                        nc.tensor.matmul(
                            ps[:],
                            xn_sb[:, 2 * i : 2 * i + 2, jt * 128 : (jt + 1) * 128],
                            t_sb[:, 2 * i : 2 * i + 2, nh * 512 : (nh + 1) * 512],
                            start=(i == 0),
                            stop=(i == 1),
                            perf_mode=DR,
                        )
                    nc.scalar.activation(
                        e_sb[:, jt, nh * 512 : (nh + 1) * 512],
                        ps[:],
                        mybir.ActivationFunctionType.Exp,
                        scale=SCALE / WSCALE,
                        bias=eshift_sb[:],
                    )
                dps = psd.tile([1, 512], F32, name=f"den{nh}", tag="psd")
                dps_tiles.append(dps)
                for jp in range(4):
                    nc.tensor.matmul(
                        dps[:],
                        ones8_sb[:, 0:2, 0:1],
                        e_sb[:, 2 * jp : 2 * jp + 2, nh * 512 : (nh + 1) * 512],
                        start=(jp == 0),
                        stop=(jp == 3),
                        perf_mode=DR,
                    )
                rsl = recip_sb[0:1, nh * 512 : (nh + 1) * 512]
                rscr = spool.tile([1, 512], F32, tag="rscr")
                nc.vector.reciprocal_approx_accurate(rsl, dps[:], rscr[:])
                nc.vector.tensor_copy(
                    recip_bf[0:1, nh * 512 : (nh + 1) * 512], rsl
                )

            # ---- broadcast 1/(16 den) across partitions (bf16 matmul) ------
            rb_sb = pers.tile([128, NPIX], F32)
            for nh in range(NH):
                bp = psd.tile([128, 512], F32, name=f"bp{nh}", tag="psd")
                nc.tensor.matmul(
                    bp[:],
                    ones_row[0:1, :],
                    recip_bf[0:1, nh * 512 : (nh + 1) * 512],
                    start=True,
                    stop=True,
                )
                nc.scalar.activation(
                    rb_sb[:, nh * 512 : (nh + 1) * 512],
                    bp[:],
                    mybir.ActivationFunctionType.Identity,
                )

            # ---- att = (16 v)^T^T @ E, normalized by 1/(16 den) ------------
            att_sb = pers.tile([128, CT, NPIX], FP8)
            for nh in range(NH):
                for ct in range(CT):
                    ps = psp.tile([128, 512], F32, tag="ps")
                    for jp in range(4):
                        nc.tensor.matmul(
                            ps[:],
                            vt_sb[:, 2 * jp : 2 * jp + 2, ct * 128 : (ct + 1) * 128],
                            e_sb[:, 2 * jp : 2 * jp + 2, nh * 512 : (nh + 1) * 512],
                            start=(jp == 0),
                            stop=(jp == 3),
                            perf_mode=DR,
                        )
                    nc.vector.tensor_mul(
                        att_sb[:, ct, nh * 512 : (nh + 1) * 512],
                        ps[:],
                        rb_sb[:, nh * 512 : (nh + 1) * 512],
                    )

            # ---- out = proj_w @ att + proj_b + x, streamed to DRAM ---------
            for nh in range(NH):
                for co in range(CT):
                    ps = psp.tile([128, 512], F32, tag="ps")
                    for i in range(2):
                        nc.tensor.matmul(
                            ps[:],
                            pw_sb[:, 2 * i : 2 * i + 2, co * 128 : (co + 1) * 128],
                            att_sb[:, 2 * i : 2 * i + 2, nh * 512 : (nh + 1) * 512],
                            start=(i == 0),
                            stop=(i == 1),
                            perf_mode=DR,
                        )
                    sl = (slice(None), co, slice(nh * 512, (nh + 1) * 512))
                    nc.vector.scalar_tensor_tensor(
                        out=x_sb[sl],
                        in0=ps[:],
                        scalar=pb_sb[:, co : co + 1],
                        in1=x_sb[sl],
                        op0=mybir.AluOpType.add,
                        op1=mybir.AluOpType.add,
                    )
                    nc.sync.dma_start(y_d[sl], x_sb[sl])

    nc.compile()
    return nc


def _build_f32r():
    """Legacy float32r build, used only when the q-bias is nonzero (the
    q/k fold is then invalid). Explicit q, k with their biases."""
    nc = bacc.Bacc("TRN2")

    x_d = nc.dram_tensor("x", [128, CT, NPIX], F32, kind="ExternalInput")
    qw_d = nc.dram_tensor("qw", [128, CT, 3 * C], F32R, kind="ExternalInput")
    pw_d = nc.dram_tensor("pw", [128, CT, C], F32R, kind="ExternalInput")
    gnw_d = nc.dram_tensor("gnw", [128, CT], F32, kind="ExternalInput")
    gnb_d = nc.dram_tensor("gnb", [128, CT], F32, kind="ExternalInput")
    qb_d = nc.dram_tensor("qb", [128, CT], F32, kind="ExternalInput")
    kb_d = nc.dram_tensor("kb", [128, CT], F32, kind="ExternalInput")
    pb_d = nc.dram_tensor("pb", [128, CT], F32, kind="ExternalInput")
    y_d = nc.dram_tensor("y", [128, CT, NPIX], F32, kind="ExternalOutput")

    ind1 = np.zeros((128, CT * G), np.float32)
    for ct in range(CT):
        for p in range(128):
            ind1[p, ct * G + ct * 8 + p // GS] = 1.0
    ind2 = np.zeros((G, C), np.float32)
    for c in range(C):
        ind2[c // GS, c] = 1.0
    ind1_d = nc.inline_tensor(ind1, name="ind1")
    ind2_d = nc.inline_tensor(ind2, name="ind2")
    onesc_d = nc.dram_tensor("onesc", [128, 512], F32R, kind="ExternalInput")
    onesr_d = nc.dram_tensor("onesr", [1, 128], F32R, kind="ExternalInput")

    with tile.TileContext(nc) as tc:
        with (
            nc.allow_low_precision(reason="float32r matmul operands"),
            tc.tile_pool(name="persist", bufs=1) as pers,
            tc.tile_pool(name="small", bufs=4) as spool,
            tc.tile_pool(name="ps", bufs=8, space="PSUM") as psp,
        ):
            onesc_sb = pers.tile([128, 512], F32R)
            nc.sync.dma_start(onesc_sb[:], onesc_d[:])
            ones_col = onesc_sb[:, 0:1]

            x_sb = pers.tile([128, CT, NPIX], F32)
            x_dmas = []
            for ct in range(CT):
                for nh in range(NH):
                    x_dmas.append(
                        nc.sync.dma_start(
                            x_sb[:, ct, nh * 512 : (nh + 1) * 512],
                            x_d[:, ct, nh * 512 : (nh + 1) * 512],
                        )
                    )

            gnw_sb = pers.tile([128, CT], F32)
            nc.sync.dma_start(gnw_sb[:], gnw_d[:])
            gnb_sb = pers.tile([128, CT], F32)
            nc.sync.dma_start(gnb_sb[:], gnb_d[:])
            ind1_sb = pers.tile([128, CT * G], F32)
            nc.sync.dma_start(ind1_sb[:], ind1_d[:])
            ind2_sb = pers.tile([G, C], F32)
            nc.sync.dma_start(ind2_sb[:], ind2_d[:])
            qb_sb = pers.tile([128, CT], F32)
            nc.sync.dma_start(qb_sb[:], qb_d[:])
            kb_sb = pers.tile([128, CT], F32)
            nc.sync.dma_start(kb_sb[:], kb_d[:])
            pb_sb = pers.tile([128, CT], F32)
            nc.sync.dma_start(pb_sb[:], pb_d[:])
            ones_row = pers.tile([1, 128], F32R)
            nc.sync.dma_start(ones_row[:], onesr_d[:])

            qw_sb = pers.tile([128, CT, 3 * C], F32R)
            for ci in range(CT):
                d = nc.sync.dma_start(qw_sb[:, ci, :], qw_d[:, ci, :])
                add_dep_helper(d.ins, x_dmas[-1].ins, sync=True,
                               reason="let x DMA finish first")
            pw_sb = pers.tile([128, CT, C], F32R)
            d = nc.sync.dma_start(pw_sb[:], pw_d[:])
            add_dep_helper(d.ins, x_dmas[-1].ins, sync=True,
                           reason="let x DMA finish first")

            eps_sb = pers.tile([G, 1], F32)
            nc.vector.memset(eps_sb[:], EPS)
            ones_row32 = pers.tile([1, 128], F32)
            nc.vector.memset(ones_row32[:], 1.0)

            warm_ps = psp.tile([128, 512], F32, tag="ps")
            for _ in range(38):
                nc.tensor.matmul(
                    warm_ps[:], onesc_sb[:, 0:128], onesc_sb[:], start=True, stop=True
                )

            statcols = pers.tile([128, CT, 2], F32)
            for ct in range(CT):
                st6 = spool.tile([128, 2, 6], F32, tag="st6")
                nc.vector.bn_stats(st6[:, 0, :], x_sb[:, ct, 0:512])
                nc.vector.bn_stats(st6[:, 1, :], x_sb[:, ct, 512:1024])
                mv = spool.tile([128, 2], F32, tag="mv")
                nc.vector.bn_aggr(mv[:], st6[:])
                nc.vector.tensor_copy(statcols[:, ct, 0:1], mv[:, 0:1])
                nc.vector.tensor_mul(statcols[:, ct, 1:2], mv[:, 0:1], mv[:, 0:1])
                nc.vector.tensor_add(
                    statcols[:, ct, 1:2], statcols[:, ct, 1:2], mv[:, 1:2]
                )

            gsum_ps = psp.tile([G, 2], F32, tag="ps")
            for ct in range(CT):
                nc.tensor.matmul(
                    gsum_ps[:],
                    ind1_sb[:, ct * G : (ct + 1) * G],
                    statcols[:, ct, :],
                    start=(ct == 0),
                    stop=(ct == CT - 1),
                )
            gs_sb = spool.tile([G, 2], F32, tag="gs")
            nc.vector.tensor_scalar_mul(gs_sb[:], gsum_ps[:], 1.0 / GS)
            var32 = spool.tile([G, 1], F32, tag="var32")
            nc.vector.tensor_mul(var32[:], gs_sb[:, 0:1], gs_sb[:, 0:1])
            nc.vector.tensor_sub(var32[:], gs_sb[:, 1:2], var32[:])
            grow = pers.tile([G, 2], F32)
            lnv = spool.tile([G, 1], F32, tag="lnv")
            nc.scalar.activation(
                lnv[:], var32[:], mybir.ActivationFunctionType.Ln, bias=eps_sb[:]
            )
            nc.scalar.activation(
                grow[:, 0:1], lnv[:], mybir.ActivationFunctionType.Exp, scale=-0.5
            )
            nc.vector.tensor_mul(grow[:, 1:2], gs_sb[:, 0:1], grow[:, 0:1])

            xn_sb = pers.tile([128, CT, NPIX], F32R)
            chsb = pers.tile([128, CT, 2], F32)
            for ct in range(CT):
                bc_ps = psp.tile([128, 2], F32, tag="ps")
                nc.tensor.matmul(
                    bc_ps[:],
                    ind2_sb[:, ct * 128 : (ct + 1) * 128],
                    grow[:],
                    start=True,
                    stop=True,
                )
                nc.vector.tensor_mul(
                    chsb[:, ct, 0:1], gnw_sb[:, ct : ct + 1], bc_ps[:, 0:1]
                )
                nc.vector.tensor_mul(
                    chsb[:, ct, 1:2], gnw_sb[:, ct : ct + 1], bc_ps[:, 1:2]
                )
                nc.vector.tensor_sub(
                    chsb[:, ct, 1:2], gnb_sb[:, ct : ct + 1], chsb[:, ct, 1:2]
                )
                nc.vector.tensor_scalar(
                    out=xn_sb[:, ct, :],
                    in0=x_sb[:, ct, :],
                    scalar1=chsb[:, ct, 0:1],
                    scalar2=chsb[:, ct, 1:2],
                    op0=mybir.AluOpType.mult,
                    op1=mybir.AluOpType.add,
                )

            q_sb = pers.tile([128, CT, NPIX], F32R)
            k_sb = pers.tile([128, CT, NPIX], F32R)
            for dst, wofs, b_sb in ((q_sb, 0, qb_sb), (k_sb, C, kb_sb)):
                for co in range(CT):
                    for nh in range(NH):
                        ps = psp.tile([128, 512], F32, tag="ps")
                        for ci in range(CT):
                            nc.tensor.matmul(
                                ps[:],
                                qw_sb[:, ci, wofs + co * 128 : wofs + (co + 1) * 128],
                                xn_sb[:, ci, nh * 512 : (nh + 1) * 512],
                                start=(ci == 0),
                                stop=(ci == CT - 1),
                            )
                        nc.scalar.activation(
                            dst[:, co, nh * 512 : (nh + 1) * 512],
                            ps[:],
                            mybir.ActivationFunctionType.Identity,
                            bias=b_sb[:, co : co + 1],
                        )

            vt_sb = pers.tile([128, JT, C], F32R)
            for jt in range(JT):
                ps = psp.tile([128, 512], F32, tag="ps")
                for ci in range(CT):
                    nc.tensor.matmul(
                        ps[:],
                        xn_sb[:, ci, jt * 128 : (jt + 1) * 128],
                        qw_sb[:, ci, 2 * C : 3 * C],
                        start=(ci == 0),
                        stop=(ci == CT - 1),
                    )
                nc.vector.tensor_copy(vt_sb[:, jt, :], ps[:])

            e_sb = pers.tile([128, JT, NPIX], F32R)
            recip_sb = pers.tile([1, NPIX], F32)
            for nh in range(NH):
                dps = psp.tile([1, 512], F32, name=f"den{nh}", tag="ps")
                for jt in range(JT):
                    ps = psp.tile([128, 512], F32, tag="ps")
                    for ci in range(CT):
                        nc.tensor.matmul(
                            ps[:],
                            k_sb[:, ci, jt * 128 : (jt + 1) * 128],
                            q_sb[:, ci, nh * 512 : (nh + 1) * 512],
                            start=(ci == 0),
                            stop=(ci == CT - 1),
                        )
                    esl = e_sb[:, jt, nh * 512 : (nh + 1) * 512]
                    nc.scalar.activation(
                        esl, ps[:], mybir.ActivationFunctionType.Exp, scale=SCALE
                    )
                    nc.tensor.matmul(
                        dps[:],
                        onesc_sb[:, 0:1],
                        esl,
                        start=(jt == 0),
                        stop=(jt == JT - 1),
                    )
                rsl = recip_sb[0:1, nh * 512 : (nh + 1) * 512]
                rscr = spool.tile([1, 512], F32, tag="rscr")
                nc.vector.reciprocal_approx_accurate(rsl, dps[:], rscr[:])

            rb_sb = pers.tile([128, NPIX], F32)
            att_sb = pers.tile([128, CT, NPIX], F32R)
            for nh in range(NH):
                bp = psp.tile([128, 512], F32, name=f"bp{nh}", tag="ps")
                nc.tensor.matmul(
                    bp[:],
                    ones_row32[0:1, :],
                    recip_sb[0:1, nh * 512 : (nh + 1) * 512],
                    start=True,
                    stop=True,
                )
                nc.scalar.activation(
                    rb_sb[:, nh * 512 : (nh + 1) * 512],
                    bp[:],
                    mybir.ActivationFunctionType.Identity,
                )
                for ct in range(CT):
                    ps = psp.tile([128, 512], F32, tag="ps")
                    for jt in range(JT):
                        nc.tensor.matmul(
                            ps[:],
                            vt_sb[:, jt, ct * 128 : (ct + 1) * 128],
                            e_sb[:, jt, nh * 512 : (nh + 1) * 512],
                            start=(jt == 0),
                            stop=(jt == JT - 1),
                        )
                    nc.vector.tensor_mul(
                        att_sb[:, ct, nh * 512 : (nh + 1) * 512],
                        ps[:],
                        rb_sb[:, nh * 512 : (nh + 1) * 512],
                    )

            for nh in range(NH):
                for co in range(CT):
                    ps = psp.tile([128, 512], F32, tag="ps")
                    for ci in range(CT):
                        nc.tensor.matmul(
                            ps[:],
                            pw_sb[:, ci, co * 128 : (co + 1) * 128],
                            att_sb[:, ci, nh * 512 : (nh + 1) * 512],
                            start=(ci == 0),
                            stop=(ci == CT - 1),
                        )
                    sl = (slice(None), co, slice(nh * 512, (nh + 1) * 512))
                    nc.vector.scalar_tensor_tensor(
                        out=x_sb[sl],
                        in0=ps[:],
                        scalar=pb_sb[:, co : co + 1],
                        in1=x_sb[sl],
                        op0=mybir.AluOpType.add,
                        op1=mybir.AluOpType.add,
                    )
                    nc.sync.dma_start(y_d[sl], x_sb[sl])

    nc.compile()
    return nc


def kernel(x, gn_weight, gn_bias, qkv_w, qkv_b, proj_w, proj_b):
    global LAST_RESULTS
    b, c, h, w = x.shape
    assert (b, c, h * w) == (8, C, NPIX)

    qkv_b = np.asarray(qkv_b, np.float32)
    qkv_w = np.asarray(qkv_w, np.float32)
    proj_w = np.asarray(proj_w, np.float32)
    # The per-query bias term cancels in softmax; a nonzero q-bias would
    # contribute a per-key term, so only then fall back to explicit q/k.
    fold_qk = not np.any(qkv_b[0:C])

    if ("nc", fold_qk) not in _cache:
        _cache[("nc", fold_qk)] = _build_fp8() if fold_qk else _build_f32r()
    nc = _cache[("nc", fold_qk)]

    def col(v):  # [512] vector -> [128, CT] per-partition columns
        return np.ascontiguousarray(np.asarray(v, np.float32).reshape(CT, 128).T)

    def wtile(wT, cols, dt=np.float32):  # [c_in, cols] -> [128, CT, cols]
        return np.ascontiguousarray(
            np.asarray(wT).astype(dt).reshape(CT, 128, cols).transpose(1, 0, 2)
        )

    if fold_qk:
        # A^T = W_q^T W_k in fp64 (so that lhsT-layout gives t = W_k^T W_q xn),
        # scaled x16 to keep e4m3 operands in the normal range.
        At = (qkv_w[0:C].astype(np.float64).T @ qkv_w[C : 2 * C].astype(np.float64))
        qw_host = np.concatenate(
            [WSCALE * At, WSCALE * qkv_w[2 * C :].T.astype(np.float64)], axis=1
        )
        shared = {
            "qw": wtile(qw_host, 2 * C, ml_dtypes.float8_e4m3fn),
            "pw": wtile(proj_w.T, C, ml_dtypes.float8_e4m3fn),
            "gnw": col(gn_weight),
            "gnb": col(gn_bias),
            # attention rows sum to 1, so att(v + b_v) = att(v) + b_v; fold the
            # v bias through proj into the proj bias on the host.
            "pb": col(proj_b + proj_w @ qkv_b[2 * C :]),
            "ones8": np.full((128, 2, 512), WSCALE, ml_dtypes.float8_e4m3fn),
            "onesr": np.ones((1, 128), ml_dtypes.bfloat16),
        }
    else:
        shared = {
            "qw": wtile(qkv_w.T, 3 * C),
            "pw": wtile(proj_w.T, C),
            "gnw": col(gn_weight),
            "gnb": col(gn_bias),
            "pb": col(proj_b + proj_w @ qkv_b[2 * C :]),
            "qb": col(qkv_b[0:C]),
            "kb": col(qkv_b[C : 2 * C]),
            "onesc": np.ones((128, 512), np.float32),
            "onesr": np.ones((1, 128), np.float32),
        }

    xs = np.asarray(x, np.float32).reshape(b, CT, 128, NPIX)
    in_maps = [
        {"x": np.ascontiguousarray(xs[i].transpose(1, 0, 2)), **shared}
        for i in range(b)
    ]

    res = run_bass_kernel_spmd(
        nc, in_maps, core_ids=list(range(8)), trace=TRACE, **TRACE_KW
    )
    LAST_RESULTS = res
    out = np.stack(
        [r["y"].transpose(1, 0, 2).reshape(c, h, w) for r in res.results]
    )
    return out.astype(np.float32)
